# revision 19
# baseline (speedup 1.0000x reference)
"""
CoAttention GNN message-passing kernel for 8x Trainium2 NeuronCores.

Reference semantics:
    k1 = node1 @ Wk.T ; k2 = node2 @ Wk.T ; v1 = node1 @ Wv.T ; v2 = node2 @ Wv.T
    t[e]  = <k1[s1[e]], k2[s2[e]]>                        (E edges)
    a1    = segment_softmax(t, s1) ; a2 = segment_softmax(t, s2)
    msg1  = segment_sum(a1 * v2[s2], s1) ; msg2 = segment_sum(a2 * v1[s1], s2)
    out_i = LeakyReLU(msg_i @ Wo.T + bo)
    returns (out1, out2, a1[:,None], a2[:,None])

Key structure: BOTH s1 and s2 arrive sorted, so the edge list is a monotone
staircase in the (s1, s2) grid.  Grouping 128 consecutive s1-segments (a
"chunk") confines that chunk's s2 values to a narrow window (<= R2CAP wide).
All per-edge math collapses to dense [128, R2CAP] blocks:

    M    = K1[chunk rows]^T-contract K2[window]           (one PE matmul)
    E1   = exp(M / T)     (ACT; the segment-max shift is dropped -- softmax is
                           shift invariant and the +eps denominator term only
                           perturbs results at the ~1e-8 level for this data)
    C1   = E1 * cnt ; norm = rowsum(C1) + eps             (one fused DVE op;
                           cnt[r1,r2] = # edges of that pair, host-built)
    A1   = E1 / norm      (cell edge-weights; host gathers per-edge outputs)
    msgT = V2[window]^T-contract (C1/norm)^T              (PE transp + matmul)
    outT = LeakyReLU(Wo-half @ msgT + bo)                 (PE + ACT + DVE)

Sharding: edges split 8 ways aligned to segment boundaries -> no cross-device
segment straddles -> no collectives.  The s2-side pass is the mirror image run
through the same code with edges re-sorted by s2 (host argsort).  The device
program is SPMD-uniform: data-dependent window offsets are resolved by the
host sending chunk-stacked node tables.
"""

import numpy as np

# ---------------------------------------------------------------------------
# Problem constants (hardcoded per the task contract)
# ---------------------------------------------------------------------------
N1 = 20000
N2 = 20000
D_IN = 128
D_OUT = 256
TEMP = float(np.sqrt(D_IN))
SLOPE = 0.01
EPS = 1e-8

NDEV = 8
SEGS_PER_CHUNK = 128   # rows per block == PSUM partition limit
R2CAP = 192            # s2-window width per chunk (multiple of 64)

_KERNEL_CACHE = {}
LAST_EXEC_NS = None
LAST_PROFILE = None


# ---------------------------------------------------------------------------
# Host-side planning (integer bookkeeping only)
# ---------------------------------------------------------------------------
def _plan_pass(sA, sB):
    """Plan one softmax side. sA must be sorted ascending."""
    E = sA.shape[0]
    cuts = [0]
    for d in range(1, NDEV):
        pos = (E * d) // NDEV
        cuts.append(int(np.searchsorted(sA, sA[pos], side="left")))
    cuts.append(E)
    cuts = np.asarray(cuts, dtype=np.int64)

    dev_alo = np.zeros(NDEV, dtype=np.int64)
    dev_nseg = np.zeros(NDEV, dtype=np.int64)
    for d in range(NDEV):
        lo, hi = cuts[d], cuts[d + 1]
        if hi > lo:
            dev_alo[d] = sA[lo]
            dev_nseg[d] = sA[hi - 1] - sA[lo] + 1
    nchunk = int(max(1, np.max((dev_nseg + SEGS_PER_CHUNK - 1) // SEGS_PER_CHUNK)))

    dev_of_edge = (np.searchsorted(cuts, np.arange(E), side="right") - 1).astype(
        np.int64
    )
    l1_row = sA - dev_alo[dev_of_edge]
    chunk_of_edge = l1_row // SEGS_PER_CHUNK
    l1 = l1_row % SEGS_PER_CHUNK

    b2 = np.zeros((NDEV, nchunk), dtype=np.int64)
    for d in range(NDEV):
        lo, hi = cuts[d], cuts[d + 1]
        if hi <= lo:
            continue
        ch = chunk_of_edge[lo:hi]
        sb = sB[lo:hi]
        bounds = np.searchsorted(ch, np.arange(nchunk + 1), side="left")
        for c in range(nchunk):
            s, e = bounds[c], bounds[c + 1]
            if e > s:
                b2[d, c] = sb[s:e].min()
    l2 = sB - b2[dev_of_edge, chunk_of_edge]
    assert l2.min() >= 0 and l2.max() < R2CAP, (
        f"chunk s2-window span {int(l2.max()) + 1} exceeds R2CAP={R2CAP}"
    )

    flat = ((dev_of_edge * nchunk + chunk_of_edge) * SEGS_PER_CHUNK + l1) * R2CAP + l2
    cnt = np.bincount(flat, minlength=NDEV * nchunk * SEGS_PER_CHUNK * R2CAP)
    cnt = cnt.reshape(NDEV, nchunk, SEGS_PER_CHUNK, R2CAP).astype(np.float32)

    return dict(
        cuts=cuts, dev_alo=dev_alo, dev_nseg=dev_nseg, nchunk=nchunk, b2=b2,
        dev_of_edge=dev_of_edge, chunk_of_edge=chunk_of_edge, l1=l1, l2=l2,
        cnt=cnt,
    )


def _stack_A(nodeAT, plan, nchunk):
    N = nodeAT.shape[1]
    out = np.zeros((NDEV, 128, nchunk * SEGS_PER_CHUNK), dtype=np.float32)
    for d in range(NDEV):
        lo = int(plan["dev_alo"][d])
        hi = min(lo + nchunk * SEGS_PER_CHUNK, N)
        if hi > lo:
            out[d, :, : hi - lo] = nodeAT[:, lo:hi]
    return out


def _stack_B(nodeBT, plan, nchunk):
    N = nodeBT.shape[1]
    out = np.zeros((NDEV, nchunk, 128, R2CAP), dtype=np.float32)
    for d in range(NDEV):
        for c in range(nchunk):
            lo = int(plan["b2"][d, c])
            hi = min(lo + R2CAP, N)
            if hi > lo:
                out[d, c, :, : hi - lo] = nodeBT[:, lo:hi]
    return out


# ---------------------------------------------------------------------------
# Device kernel builder (Bass / Tile)
# ---------------------------------------------------------------------------
def _build_nc(nchunk1, nchunk2, stage=99):
    import os
    from contextlib import ExitStack

    import concourse.bacc as bacc
    import concourse.mybir as mybir
    import concourse.tile as tile

    fp32 = mybir.dt.float32
    bf16 = mybir.dt.bfloat16
    AF = mybir.ActivationFunctionType
    ALU = mybir.AluOpType

    nc = bacc.Bacc("TRN2", target_bir_lowering=False, debug=False,
                   num_devices=NDEV)

    def dparam(name, shape, dtype=fp32, out=False):
        return nc.declare_dram_parameter(name, list(shape), dtype, isOutput=out)[:]

    wkT = dparam("wkT", (D_IN, D_IN))
    wvT = dparam("wvT", (D_IN, D_IN))
    woT = dparam("woT", (D_IN, D_OUT))
    bo_d = dparam("bo", (128, 2))
    ident = dparam("ident", (128, 128))

    passes = []
    for p, nchunk in ((1, nchunk1), (2, nchunk2)):
        passes.append(dict(
            nchunk=nchunk,
            nAT=dparam(f"nAT{p}", (128, nchunk * SEGS_PER_CHUNK)),
            nBT=dparam(f"nBT{p}", (nchunk, 128, R2CAP)),
            cnt=dparam(f"cnt{p}", (nchunk, 128, R2CAP), bf16),
            aout=dparam(f"aout{p}", (nchunk, 128, R2CAP), out=True),
            oout=dparam(f"oout{p}", (nchunk, 2, 128, 128), out=True),
        ))

    with tile.TileContext(nc) as tc, ExitStack() as ctx:
        cpool = ctx.enter_context(tc.tile_pool(name="consts", bufs=1))
        wkT_sb = cpool.tile([D_IN, D_IN], fp32, tag="wk")
        nc.sync.dma_start(wkT_sb[:], wkT)
        wvT_sb = cpool.tile([D_IN, D_IN], fp32, tag="wv")
        nc.sync.dma_start(wvT_sb[:], wvT)
        woT_sb = cpool.tile([D_IN, D_OUT], fp32, tag="wo")
        nc.sync.dma_start(woT_sb[:], woT)
        bo_sb = cpool.tile([128, 2], fp32, tag="bo")
        nc.sync.dma_start(bo_sb[:], bo_d)
        id_sb = cpool.tile([128, 128], fp32, tag="id")
        nc.sync.dma_start(id_sb[:], ident)

        for P in passes:
            nchunk = P["nchunk"]
            ncols = nchunk * SEGS_PER_CHUNK
            with ExitStack() as pctx:
                tp = pctx.enter_context(tc.tile_pool(name="tables", bufs=1))
                nAT_sb = tp.tile([128, ncols], fp32, tag="nAT")
                nc.sync.dma_start(nAT_sb[:], P["nAT"])
                kAT = tp.tile([128, ncols], fp32, tag="kAT")
                kBT = tp.tile([128, nchunk * R2CAP], fp32, tag="kBT")
                v0 = tp.tile([128, nchunk * 128], fp32, tag="v0")
                v1 = tp.tile([64, nchunk * 128], fp32, tag="v1")

                # ---- table build ----
                with tc.tile_pool(name="tbuild", bufs=2, space="PSUM") as pb, \
                     tc.tile_pool(name="tbin", bufs=3) as bin_pool:
                    for j in range(0, ncols, 512):
                        w = min(512, ncols - j)
                        ps = pb.tile([128, 512], fp32, tag="ka")
                        nc.tensor.matmul(ps[:, :w], wkT_sb[:],
                                         nAT_sb[:, j:j + w],
                                         start=True, stop=True)
                        nc.scalar.copy(kAT[:, j:j + w], ps[:, :w])
                    for c in range(nchunk):
                        nb = bin_pool.tile([128, R2CAP], fp32, tag="nb")
                        nc.sync.dma_start(nb[:], P["nBT"][c])
                        ps = pb.tile([128, R2CAP], fp32, tag="kb")
                        nc.tensor.matmul(ps[:], wkT_sb[:], nb[:],
                                         start=True, stop=True)
                        nc.scalar.copy(kBT[:, c * R2CAP:(c + 1) * R2CAP], ps[:])
                        pv0 = pb.tile([128, 128], fp32, tag="pv0")
                        nc.tensor.matmul(pv0[:], nb[:, 0:128], wvT_sb[:],
                                         start=True, stop=True)
                        nc.scalar.copy(v0[:, c * 128:(c + 1) * 128], pv0[:])
                        pv1 = pb.tile([64, 128], fp32, tag="pv1")
                        nc.tensor.matmul(pv1[:], nb[:, 128:R2CAP], wvT_sb[:],
                                         start=True, stop=True)
                        nc.scalar.copy(v1[:, c * 128:(c + 1) * 128], pv1[:])

                # ---- chunk loop ----
                with tc.tile_pool(name="pm", bufs=2, space="PSUM") as pm, \
                     tc.tile_pool(name="ptr", bufs=2, space="PSUM") as ptr, \
                     tc.tile_pool(name="pmsg", bufs=2, space="PSUM") as pmsg, \
                     tc.tile_pool(name="po", bufs=2, space="PSUM") as po, \
                     tc.tile_pool(name="work", bufs=3) as wk, \
                     tc.tile_pool(name="small", bufs=4) as sm:
                    for c in range(nchunk):
                        kA = kAT[:, c * 128:(c + 1) * 128]
                        kB = kBT[:, c * R2CAP:(c + 1) * R2CAP]
                        psM = pm.tile([128, R2CAP], fp32, tag="M")
                        nc.tensor.matmul(psM[:], kA, kB, start=True, stop=True)

                        cnt_sb = wk.tile([128, R2CAP], bf16, tag="cnt")
                        nc.sync.dma_start(cnt_sb[:], P["cnt"][c])
                        cnt_f = wk.tile([128, R2CAP], fp32, tag="cntf")
                        nc.vector.tensor_copy(cnt_f[:], cnt_sb[:])

                        e1 = wk.tile([128, R2CAP], fp32, tag="e1")
                        nc.scalar.activation(e1[:], psM[:], AF.Exp,
                                             scale=1.0 / TEMP)
                        if stage < 2:
                            nc.sync.dma_start(P["aout"][c], e1[:])
                            continue
                        c1 = wk.tile([128, R2CAP], fp32, tag="c1")
                        nc.vector.tensor_mul(c1[:], e1[:], cnt_f[:])
                        norm = sm.tile([128, 1], fp32, tag="norm")
                        nc.vector.tensor_reduce(norm[:], c1[:],
                                                mybir.AxisListType.X, ALU.add)
                        norme = sm.tile([128, 1], fp32, tag="norme")
                        nc.vector.tensor_scalar_add(norme[:], norm[:], EPS)
                        rec = sm.tile([128, 1], fp32, tag="rec")
                        nc.vector.reciprocal(rec[:], norme[:])
                        a1 = wk.tile([128, R2CAP], fp32, tag="a1")
                        nc.vector.tensor_scalar_mul(a1[:], e1[:], rec[:])
                        nc.sync.dma_start(P["aout"][c], a1[:])
                        if stage < 3:
                            continue
                        a1c = wk.tile([128, R2CAP], fp32, tag="a1c")
                        nc.vector.tensor_scalar_mul(a1c[:], c1[:], rec[:])

                        t0p = ptr.tile([128, 128], fp32, tag="t")
                        nc.tensor.transpose(t0p[:], a1c[:, 0:128], id_sb[:])
                        t1p = ptr.tile([64, 128], fp32, tag="t")
                        nc.tensor.transpose(t1p[:], a1c[:, 128:R2CAP], id_sb[:])
                        t0 = sm.tile([128, 128], fp32, tag="t0s")
                        nc.scalar.copy(t0[:], t0p[:])
                        t1 = sm.tile([64, 128], fp32, tag="t1s")
                        nc.scalar.copy(t1[:], t1p[:])
                        if stage < 4:
                            continue

                        msgp = pmsg.tile([128, 128], fp32, tag="msg")
                        nc.tensor.matmul(msgp[:], v0[:, c * 128:(c + 1) * 128],
                                         t0[:], start=True, stop=False)
                        nc.tensor.matmul(msgp[:], v1[:, c * 128:(c + 1) * 128],
                                         t1[:], start=False, stop=True)
                        msgT = sm.tile([128, 128], fp32, tag="msgs")
                        nc.scalar.copy(msgT[:], msgp[:])
                        if stage < 5:
                            continue

                        for h in range(2):
                            op = po.tile([128, 128], fp32, tag="o")
                            nc.tensor.matmul(
                                op[:], woT_sb[:, h * 128:(h + 1) * 128],
                                msgT[:], start=True, stop=True,
                            )
                            yb = sm.tile([128, 128], fp32, tag="yb")
                            nc.scalar.activation(yb[:], op[:], AF.Identity,
                                                 bias=bo_sb[:, h:h + 1])
                            ys = sm.tile([128, 128], fp32, tag="ys")
                            nc.vector.tensor_scalar_mul(ys[:], yb[:], SLOPE)
                            ob = sm.tile([128, 128], fp32, tag="ob")
                            nc.vector.tensor_max(ob[:], yb[:], ys[:])
                            nc.sync.dma_start(P["oout"][c, h], ob[:])
    nc.compile()
    return nc


# ---------------------------------------------------------------------------
# Top-level entry
# ---------------------------------------------------------------------------
def kernel(node1, seg_i1, idx_j1, node2, seg_i2, idx_j2, Wk, Wv, Wo, bo):
    import ml_dtypes
    from concourse.bass_utils import run_bass_kernel_spmd

    node1 = np.asarray(node1, dtype=np.float32)
    node2 = np.asarray(node2, dtype=np.float32)
    s1 = np.asarray(seg_i1, dtype=np.int64)
    s2 = np.asarray(seg_i2, dtype=np.int64)
    Wk = np.asarray(Wk, np.float32)
    Wv = np.asarray(Wv, np.float32)
    Wo = np.asarray(Wo, np.float32)
    bo = np.asarray(bo, np.float32)

    n1t = np.ascontiguousarray(node1.T)
    n2t = np.ascontiguousarray(node2.T)

    plan1 = _plan_pass(s1, s2)
    perm = np.argsort(s2, kind="stable")
    plan2 = _plan_pass(s2[perm], s1[perm])
    nchunk1, nchunk2 = plan1["nchunk"], plan2["nchunk"]

    import os
    stage = int(os.environ.get("KSTAGE", "99"))
    key = (nchunk1, nchunk2, stage)
    if key not in _KERNEL_CACHE:
        _KERNEL_CACHE[key] = _build_nc(nchunk1, nchunk2, stage)
    nc = _KERNEL_CACHE[key]

    nAT1 = _stack_A(n1t, plan1, nchunk1)
    nBT1 = _stack_B(n2t, plan1, nchunk1)
    nAT2 = _stack_A(n2t, plan2, nchunk2)
    nBT2 = _stack_B(n1t, plan2, nchunk2)

    wkT = np.ascontiguousarray(Wk.T)
    wvT = np.ascontiguousarray(Wv.T)
    woT = np.ascontiguousarray(Wo.T)
    bo2 = np.ascontiguousarray(bo.reshape(2, 128).T)
    ident = np.eye(128, dtype=np.float32)

    in_maps = []
    for d in range(NDEV):
        in_maps.append(dict(
            wkT=wkT, wvT=wvT, woT=woT, bo=bo2, ident=ident,
            nAT1=nAT1[d], nBT1=np.ascontiguousarray(nBT1[d]),
            cnt1=plan1["cnt"][d].astype(ml_dtypes.bfloat16),
            nAT2=nAT2[d], nBT2=np.ascontiguousarray(nBT2[d]),
            cnt2=plan2["cnt"][d].astype(ml_dtypes.bfloat16),
        ))

    trace = bool(int(os.environ.get("KPROF", "0")))
    if trace and "antenv.axon_hooks" not in __import__("sys").modules:
        import sys as _sys
        import types as _types
        from trn_agent_boot.trn_boot import _ntff_profile_via_ctypes
        _m = _types.ModuleType("antenv.axon_hooks")
        _h = _ntff_profile_via_ctypes("/opt/axon/libaxon_pjrt.so")
        _m.get_axon_ntff_profile_hook = lambda: _h
        _sys.modules["antenv.axon_hooks"] = _m
    res = run_bass_kernel_spmd(nc, in_maps, list(range(NDEV)), trace=trace)
    results = res.results
    global LAST_EXEC_NS, LAST_PROFILE
    LAST_EXEC_NS = res.exec_time_ns
    LAST_PROFILE = res.profile_json

    def assemble(plan, nchunk, key_o, key_a, nseg_total):
        out = np.empty((nseg_total, D_OUT), dtype=np.float32)
        bias_row = np.where(bo >= 0, bo, SLOPE * bo).astype(np.float32)
        out[:] = bias_row[None, :]
        a_blocks = np.stack([results[d][key_a] for d in range(NDEV)])
        for d in range(NDEV):
            ns = int(plan["dev_nseg"][d])
            if ns == 0:
                continue
            lo = int(plan["dev_alo"][d])
            ot = results[d][key_o]  # [nchunk, 2, 128, 128]
            dense = ot.transpose(0, 3, 1, 2).reshape(nchunk * 128, D_OUT)
            out[lo:lo + ns] = dense[:ns]
        edge = a_blocks[
            plan["dev_of_edge"], plan["chunk_of_edge"], plan["l1"], plan["l2"]
        ].astype(np.float32)
        return out, edge

    out1, edge1 = assemble(plan1, nchunk1, "oout1", "aout1", node1.shape[0])
    out2, edge2s = assemble(plan2, nchunk2, "oout2", "aout2", node2.shape[0])
    edge2 = np.empty_like(edge2s)
    edge2[perm] = edge2s

    return out1, out2, edge1[:, None], edge2[:, None]
